# revision 1
# baseline (speedup 1.0000x reference)
"""AlphaFold-style OuterProductMean pair feature on 8 trn2 NeuronCores.

Computation (full shapes):
    x_left, x_right: (1, N=128, R=256, E=32) fp32
    outer[b,i,j,l,r] = sum_n x_left[b,n,i,l] * x_right[b,n,j,r]
    out = outer.reshape(1, R, R, E*E) @ W + b          # W: (1024, 128)

Sharding: row-shard the pair grid — core k owns i in [32k, 32k+32).
Each core receives its x_left row block, the full x_right, W, b
(all staged host-side; no collectives), and writes its (32, 256, 128)
output row block. Host concatenates.

Per-core kernel:
  stage 1 (bf16 matmuls, 1 cycle/row): for each i, r:
           outer_i[l, j] = xl[:, i, :].T @ xr[:, :, r]   (K=n=128)
           Four r's run concurrently via 4x column tiling (M=32 each)
           producing PSUM chunks (128 part = (r%4, l), 256 j) whose
           partition index matches rows r*32+l of the host-permuted W.
  stage 2: pair[d, (u j)] = sum_c Wp_chunk[c].T @ outer_chunk[c]
           (8 accumulating matmuls, K=128, N=512 = two i's of 256 j).
Output per core is (i, d, j); host transposes to (i, j, d).
"""

import os
import sys

if "/opt/trn_rl_repo" not in sys.path:
    sys.path.insert(0, "/opt/trn_rl_repo")

# The NTFF profile hook modules (antenv.axon_hooks / axon.trn) don't exist in
# this container; run_bass_kernel_spmd(trace=...) would crash trying them.
os.environ["BASS_NEVER_TRACE"] = "1"

from contextlib import ExitStack

import numpy as np

import concourse.bass as bass
import concourse.tile as tile
from concourse import bacc, mybir
from concourse.bass_utils import run_bass_kernel_spmd

N_CORES = 8
N = 128  # MSA depth (contraction dim)
R = 256  # residues
E = 32   # 1D embedding
D = 128  # 2D embedding
IB = R // N_CORES  # 32 rows of i per core
BENCH_REPS = 5
STAGE2_BF16 = True  # False -> float32r stage 2 (more precise, maybe slower)

_cached = None
last_results = None  # BassKernelResults of the most recent run (for test harness)


def _build(reps=1, stage2_bf16=STAGE2_BF16):
    f32 = mybir.dt.float32
    f32r = mybir.dt.float32r
    bf16 = mybir.dt.bfloat16
    s2dt = bf16 if stage2_bf16 else f32r

    nc = bacc.Bacc(None, target_bir_lowering=False, debug=False)

    xl_d = nc.dram_tensor("xl", [N, IB * E], bf16, kind="ExternalInput")    # [n, i*32+l]
    xr_d = nc.dram_tensor("xr", [N, E * R], bf16, kind="ExternalInput")     # [n, r*256+j]
    wp_d = nc.dram_tensor("wp", [D, 8 * D], s2dt, kind="ExternalInput")     # [p, c*128+d]
    out_d = nc.dram_tensor("out", [IB, D, R], f32, kind="ExternalOutput")   # [i, d, j]

    with tile.TileContext(nc) as tc, ExitStack() as ctx:
        const = ctx.enter_context(tc.tile_pool(name="const", bufs=1))
        xl_sb = const.tile([N, IB * E], bf16)
        xr_sb = const.tile([N, E * R], bf16)
        wp_sb = const.tile([D, 8 * D], s2dt)

        nc.sync.dma_start(xl_sb[:], xl_d[:])
        for q in range(8):
            s = q * (E * R // 8)
            w = E * R // 8
            nc.sync.dma_start(xr_sb[:, s:s + w], xr_d[:, s:s + w])
        nc.sync.dma_start(wp_sb[:], wp_d[:])

        outer_pool = ctx.enter_context(tc.tile_pool(name="outer", bufs=3))
        ps1 = ctx.enter_context(tc.tile_pool(name="ps1", bufs=6, space="PSUM"))
        ps2 = ctx.enter_context(tc.tile_pool(name="ps2", bufs=2, space="PSUM"))
        osb_pool = ctx.enter_context(tc.tile_pool(name="osb", bufs=4))

        evac_idx = 0
        for _rep in range(reps):
            for ip in range(IB // 2):  # pairs of i rows
                outer = outer_pool.tile([D, 8, 2, R], s2dt)  # (p, c, u, j)
                for u in range(2):
                    i = 2 * ip + u
                    for cp in range(4):  # chunk pairs share one PSUM bank
                        p1 = ps1.tile([D, 2, R], f32)
                        for q in range(2):
                            c = 2 * cp + q
                            for g in range(4):
                                r0 = 4 * c + g
                                nc.tensor.matmul(
                                    p1[32 * g:32 * g + 32, q, :],
                                    xl_sb[:, E * i:E * i + E],
                                    xr_sb[:, R * r0:R * r0 + R],
                                    start=True,
                                    stop=True,
                                    tile_position=(0, 32 * g),
                                )
                        # split PSUM evacuation across DVE and ACT (5:3 —
                        # DVE ~533ns/tile also carries the bias adds, ACT
                        # ~2x slower per copy; nc.any routes all to ACT)
                        dst = outer[:, 2 * cp:2 * cp + 2, u, :]
                        if evac_idx % 3 < 2:
                            nc.vector.tensor_copy(dst, p1[:])
                        else:
                            nc.scalar.copy(dst, p1[:])
                        evac_idx += 1

                p2 = ps2.tile([D, 2, R], f32)
                for c in range(8):
                    nc.tensor.matmul(
                        p2[:],
                        wp_sb[:, D * c:D * c + D],
                        outer[:, c],
                        start=(c == 0),
                        stop=(c == 7),
                    )
                # bias is added host-side (free); plain copy to SBUF staging,
                # sharing the DVE/ACT 2:1 rotation with the stage-1 evacs
                osb = osb_pool.tile([D, 2, R], f32)
                if evac_idx % 3 < 2:
                    nc.vector.tensor_copy(osb[:], p2[:])
                else:
                    nc.scalar.copy(osb[:], p2[:])
                evac_idx += 1
                nc.sync.dma_start(out_d[2 * ip], osb[:, 0, :])
                nc.sync.dma_start(out_d[2 * ip + 1], osb[:, 1, :])

    nc.compile()
    return nc


def make_in_maps(x_left, x_right, W, b, stage2_bf16=STAGE2_BF16):
    import ml_dtypes

    xl = np.asarray(x_left, dtype=np.float32)[0]   # (n, i, l)
    xr = np.asarray(x_right, dtype=np.float32)[0]  # (n, j, r)
    W = np.asarray(W, dtype=np.float32)
    b = np.asarray(b, dtype=np.float32)

    xl = np.ascontiguousarray(xl).astype(ml_dtypes.bfloat16)
    xr_flat = np.ascontiguousarray(
        xr.transpose(0, 2, 1).astype(ml_dtypes.bfloat16)
    ).reshape(N, E * R)  # [n, r*256+j]
    # W[(l*32+r), d] -> W_perm[(r*32+l), d] -> chunk-major sbuf layout [p, c*128+d]
    wp = (
        W.reshape(E, E, D).transpose(1, 0, 2).reshape(8, D, D)
        .transpose(1, 0, 2).reshape(D, 8 * D)
    )
    wp = np.ascontiguousarray(wp)
    if stage2_bf16:
        wp = wp.astype(ml_dtypes.bfloat16)

    in_maps = []
    for k in range(N_CORES):
        xlk = np.ascontiguousarray(xl[:, IB * k:IB * (k + 1), :]).reshape(N, IB * E)
        in_maps.append({"xl": xlk, "xr": xr_flat, "wp": wp})
    return in_maps


def kernel(x_left, x_right, W, b):
    global _cached, last_results
    if _cached is None:
        _cached = _build()
    nc = _cached

    in_maps = make_in_maps(x_left, x_right, W, b)
    res = run_bass_kernel_spmd(nc, in_maps, list(range(N_CORES)))
    last_results = res

    blocks = [res.results[k]["out"].transpose(0, 2, 1) for k in range(N_CORES)]
    out = np.concatenate(blocks, axis=0)[None]  # (1, 256, 256, 128)
    out += np.asarray(b, dtype=np.float32)  # bias broadcast over d (host-side)
    return out



# revision 3
# speedup vs baseline: 3.7441x; 3.7441x over previous
"""AlphaFold-style OuterProductMean pair feature on 8 trn2 NeuronCores.

Computation (full shapes):
    x_left, x_right: (1, N=128, R=256, E=32) fp32
    outer[b,i,j,l,r] = sum_n x_left[b,n,i,l] * x_right[b,n,j,r]
    out = outer.reshape(1, R, R, E*E) @ W + b          # W: (1024, 128)

Sharding: row-shard the pair grid - core k owns i in [32k, 32k+32).
Each core gets its x_left row block, the full x_right, W (all staged
host-side; no collectives), and writes its (32, 256, 128) output row
block as [d, i, j] bf16. Host reassembles/transposes and adds bias.

Per-core kernel, two phases per rep:

Phase A (full 128x128 matmuls, M=128):
    For each quad q of 4 i-rows: lhsT = xl[n, (i4, l)] (128 cols),
    rhs = xr[n, (r, j)] -> psum[(i4,l), (r2, j)] f=512 x 16 chunks.
    PSUM evacuated to SBUF bf16 `outer` tile [(i4,l), (r, q, j)] by
    DVE/ACT alternating copies (FD=2048 each, 4 PSUM banks).

Phase B (4x row-tiled matmuls, K=32, concurrent):
    Row tile t handles i_sub=t, contracting l (32 partitions) for one
    r0 at a time, accumulating over r0=0..31 into psum[d, (q, j)].
    Weights = W[(l,r0), d] slices, replicated per partition quadrant.
    2 passes x 4 quads each fill all 8 PSUM banks; evac as [d,(i,j)].
"""

import os
import sys

if "/opt/trn_rl_repo" not in sys.path:
    sys.path.insert(0, "/opt/trn_rl_repo")

# The NTFF profile hook modules (antenv.axon_hooks / axon.trn) don't exist in
# this container; run_bass_kernel_spmd(trace=...) would crash trying them.
os.environ["BASS_NEVER_TRACE"] = "1"

from contextlib import ExitStack

import numpy as np

import concourse.bass as bass
import concourse.tile as tile
from concourse import bacc, mybir
from concourse.bass_utils import run_bass_kernel_spmd

N_CORES = 8
N = 128  # MSA depth (contraction dim)
R = 256  # residues
E = 32   # 1D embedding
D = 128  # 2D embedding
IB = R // N_CORES  # 32 rows of i per core
NQ = IB // 4       # 8 quads of 4 i-rows

_cached = None
last_results = None  # BassKernelResults of the most recent run (for test harness)


def _build(reps=1):
    f32 = mybir.dt.float32
    bf16 = mybir.dt.bfloat16

    nc = bacc.Bacc(None, target_bir_lowering=False, debug=False)

    xl_d = nc.dram_tensor("xl", [N, IB * E], bf16, kind="ExternalInput")   # [n, (i,l)]
    xr_d = nc.dram_tensor("xr", [N, E, R], bf16, kind="ExternalInput")    # [n, r, j]
    wp_d = nc.dram_tensor("wp", [D, E, D], bf16, kind="ExternalInput")    # [(t,l), r0, d]
    out_d = nc.dram_tensor("out", [D, 2, 4, 4, R], bf16, kind="ExternalOutput")
    # out layout [d, pass, qq, t, j] -> i_local = 16*pass + 4*qq + t

    with tile.TileContext(nc) as tc, ExitStack() as ctx:
        const = ctx.enter_context(tc.tile_pool(name="const", bufs=1))
        xl_sb = const.tile([N, IB * E], bf16)
        xr_sb = const.tile([N, E, R], bf16)
        wp_sb = const.tile([D, E, D], bf16)

        nc.sync.dma_start(xl_sb[:], xl_d[:])
        for c in range(8):
            nc.sync.dma_start(xr_sb[:, 4 * c:4 * c + 4, :], xr_d[:, 4 * c:4 * c + 4, :])
        for c in range(2):
            nc.sync.dma_start(wp_sb[:, 16 * c:16 * c + 16, :], wp_d[:, 16 * c:16 * c + 16, :])

        big = ctx.enter_context(tc.tile_pool(name="big", bufs=1))
        outer = big.tile([128, E, NQ, R], bf16)  # [(i4,l), r, q, j]

        osb_pool = ctx.enter_context(tc.tile_pool(name="osb", bufs=2))

        evac = 0
        for _rep in range(reps):
            # ---- Phase A: outer product, full-width matmuls ----
            with tc.tile_pool(name="psA", bufs=2, space="PSUM") as psA:
                for q in range(NQ):
                    lhsT = xl_sb[:, 128 * q:128 * q + 128]
                    for g in range(4):
                        pa = psA.tile([128, 8, R], f32, name="pa")
                        for ml in range(4):
                            m = 4 * g + ml
                            nc.tensor.matmul(
                                pa[:, 2 * ml:2 * ml + 2, :],
                                lhsT,
                                xr_sb[:, 2 * m:2 * m + 2, :],
                                start=True,
                                stop=True,
                            )
                        dst = outer[:, 8 * g:8 * g + 8, q, :]
                        if evac % 2 == 0:
                            nc.scalar.copy(dst, pa[:])
                        else:
                            nc.vector.tensor_copy(dst, pa[:])
                        evac += 1

            # ---- Phase B: project with W, 4x concurrent row tiles ----
            with tc.tile_pool(name="psB", bufs=4, space="PSUM") as psB:
                for p_ in range(2):
                    pbs = [
                        psB.tile([128, 4, R], f32, name="pb")
                        for t in range(4)
                    ]
                    for r0 in range(E):
                        for h in range(2):
                            for t in range(4):
                                nc.tensor.matmul(
                                    pbs[t][:, 2 * h:2 * h + 2, :],
                                    wp_sb[32 * t:32 * t + 32, r0, :],
                                    outer[32 * t:32 * t + 32, r0,
                                          4 * p_ + 2 * h:4 * p_ + 2 * h + 2, :],
                                    start=(r0 == 0),
                                    stop=(r0 == E - 1),
                                    tile_position=(32 * t, 0),
                                    skip_group_check=True,
                                )
                    osb = osb_pool.tile([D, 4, 4, R], bf16, name="osb")
                    for t in range(4):
                        dst = osb[:, :, t, :]
                        if evac % 2 == 0:
                            nc.scalar.copy(dst, pbs[t][:])
                        else:
                            nc.vector.tensor_copy(dst, pbs[t][:])
                        evac += 1
                    nc.sync.dma_start(out_d[:, p_], osb[:])

    nc.compile()
    return nc


def make_in_maps(x_left, x_right, W, b):
    import ml_dtypes

    xl = np.asarray(x_left, dtype=np.float32)[0]   # (n, i, l)
    xr = np.asarray(x_right, dtype=np.float32)[0]  # (n, j, r)
    W = np.asarray(W, dtype=np.float32)

    xl_f = np.ascontiguousarray(xl).astype(ml_dtypes.bfloat16).reshape(N, R * E)
    xr_f = np.ascontiguousarray(xr.transpose(0, 2, 1)).astype(ml_dtypes.bfloat16)  # [n, r, j]
    # wp[(t,l), r0, d] = W[l*E + r0, d], replicated over t
    wp = np.tile(W.reshape(E, E * D), (4, 1)).reshape(D, E, D)
    wp = np.ascontiguousarray(wp).astype(ml_dtypes.bfloat16)

    in_maps = []
    for k in range(N_CORES):
        xlk = np.ascontiguousarray(xl_f[:, IB * E * k:IB * E * (k + 1)])
        in_maps.append({"xl": xlk, "xr": xr_f, "wp": wp})
    return in_maps


def kernel(x_left, x_right, W, b):
    global _cached, last_results
    if _cached is None:
        _cached = _build()
    nc = _cached

    in_maps = make_in_maps(x_left, x_right, W, b)
    res = run_bass_kernel_spmd(nc, in_maps, list(range(N_CORES)))
    last_results = res

    blocks = []
    for k in range(N_CORES):
        blk = np.asarray(res.results[k]["out"], dtype=np.float32)  # [d, p, qq, t, j]
        blk = blk.reshape(D, IB, R).transpose(1, 2, 0)             # (i_local, j, d)
        blocks.append(blk)
    out = np.concatenate(blocks, axis=0)[None]  # (1, 256, 256, 128)
    out += np.asarray(b, dtype=np.float32)  # bias broadcast over d (host-side)
    return out
